# revision 62
# baseline (speedup 1.0000x reference)
"""Trainium2 Bass kernel for nn_Attention_55293408968939.

Full-input contract: kernel(**inputs) takes the unsharded inputs and returns
the full [1, 2048, 2048] output. Internally: 16 heads are sharded 2-per-core
across 8 NeuronCores (tensor parallel); each core computes QKV projection for
its heads, RMSNorm+3D-RoPE, non-causal attention, and its partial output
projection; the host sums the 8 partials and adds the (folded) bias row.

bf16 datapath (all matmul operands bf16, fp32 PSUM accumulation; rel-err
budget is 2e-2, this path lands ~6e-3). All DRAM inputs are host-pre-blocked
to match SBUF layout so each resident tensor loads in O(1) large DMAs (HWDGE
descriptor-gen at ~0.6us/DMA was a phase-1 bottleneck with per-tile DMAs).

  phase 1: qT/kT computed transposed [head_dim, tok] straight from the matmul
           (lhsT = w chunk, rhs = xT chunk); v computed natural [tok, head_dim]
           (lhsT = xT chunk, rhs = wvT chunk). RMS factor r = exp(-0.5*ln(mean
           sq + eps)) via GPSIMD partition all-reduce + full-tile ACT Ln/Exp
           (no partition broadcasts: Ln/Exp on [128,512] cost the same as one
           row). RoPE applied in the transposed layout with host-folded
           cos/sin tables (norm weight + pair signs + the D^-1/4 attention-
           scale split folded in) using a quadrant-local de-interleave so the
           pair swap is a stream_shuffle (+-16 in each 32-partition quadrant).
           x chunks are prefetched one token-chunk ahead in single DMAs.
  boundary: the last token chunk runs f-outer with k tensors first (ring
           PSUM for the k blocks, then a handed-over 8-bank layout for the q
           blocks + v); the (h0/h1, q-chunk-0) score+exp prelude is woven
           between those blocks so ACT streams 16 exps while PE finishes the
           projections - attention then opens PV-only with zero warmup. The
           q-tensor RMS/rope chains of chunks 2-3 are deferred past the
           boundary (first read at attention q-chunks 2-3).
  phase 2: per (head, 512-token q chunk): ST[k,q] tiles computed 2-at-a-time
           into [128,1024] PSUM tiles (2 banks), exp'd in one wide ACT op to
           bf16 E tiles, PV lagging one tile so the in-order PE queue never
           parks on an unfinished exp. Softmax denominators via a bf16 DVE
           running sum over the 8 wide E tiles + one GPSIMD partition
           all-reduce (NO PE ones-matmuls - saves 65536 PE rows/core =
           27us), then a fast DVE reciprocal and a normalize multiply into
           bf16 ctxT. The final q chunk runs in 256-token halves so the last
           projection block starts half a chunk earlier.
  phase 3: partial = ctxT.T @ proj_wT slice (bf16), interleaved one q-chunk
           behind attention one (mt,oc) unit per attention wide-tile so the
           PSUM drains (split DVE/ACT; GPSIMD cannot read PSUM) never queue
           behind a full q-chunk of sum work; one out-DMA per 128-token row
           block (two half-row DMAs for the last block so the tail is short).

Host folds: qkv v-bias contributes exactly bias_v @ proj_w.T to the output
(softmax rows sum to 1), so it is added host-side with proj_b.
"""
import sys

sys.path.insert(0, "/opt/trn_rl_repo")

import numpy as np
import ml_dtypes

NUM_HEADS = 16
N_CORES = 8
D = 128           # head dim
N = 2048          # tokens
C = 2048          # model dim
EPS = 1e-6
ROPE_THETA = 10000.0
NPBF16 = ml_dtypes.bfloat16

_CACHE = {}


def _perm_quadrant():
    """Partition permutation: quadrant b lanes 0-15 = even dims of [32b,32b+32),
    lanes 16-31 = odd dims. perm[p] = original head-dim index stored at lane p."""
    perm = np.empty(128, np.int64)
    for b in range(4):
        for j in range(16):
            perm[32 * b + j] = 32 * b + 2 * j
            perm[32 * b + 16 + j] = 32 * b + 2 * j + 1
    return perm


def _rope_tables(T, H, W, head_dim):
    dh = 2 * ((head_dim // 3) // 2)
    dw = dh
    dt = head_dim - dh - dw

    def axis_ang(L, d):
        inv = 1.0 / (ROPE_THETA ** (np.arange(0, d, 2, dtype=np.float32) / d))
        return np.arange(L, dtype=np.float32)[:, None] * inv[None, :]

    at = axis_ang(T, dt)
    ah = axis_ang(H, dh)
    aw = axis_ang(W, dw)
    at_g = np.broadcast_to(at[:, None, None, :], (T, H, W, dt // 2))
    ah_g = np.broadcast_to(ah[None, :, None, :], (T, H, W, dh // 2))
    aw_g = np.broadcast_to(aw[None, None, :, :], (T, H, W, dw // 2))
    ang = np.concatenate([at_g, ah_g, aw_g], axis=-1).reshape(T * H * W, head_dim // 2)
    return np.cos(ang), np.sin(ang)  # [N, 64] fp32


def _folded_tables(cos, sin, w, perm):
    """cosT/sinT [128, N] in the quadrant-deinterleaved transposed layout with
    norm weight and rotation signs folded in.

    lane p holds dim d = perm[p], pair index i = d // 2.
    m1 coeff at lane p = cos_i * w[d].
    After the +-16 quadrant shuffle, lane p holds the partner dim value, so
    m2 coeff = -sin_i * w[d+1] for even d, +sin_i * w[d-1] for odd d."""
    n = cos.shape[0]
    cosT = np.empty((128, n), np.float32)
    sinT = np.empty((128, n), np.float32)
    for p in range(128):
        d = int(perm[p])
        i = d // 2
        cosT[p] = cos[:, i] * w[d]
        if d % 2 == 0:
            sinT[p] = -sin[:, i] * w[d + 1]
        else:
            sinT[p] = sin[:, i] * w[d - 1]
    return cosT, sinT


def _block_rows(a, n_blk):
    """[n_blk*128, F] -> [128, n_blk, F] so row p holds block-row p of every
    128-row block (matches SBUF [128, n_blk, F] tile layout)."""
    f = a.shape[1]
    return np.ascontiguousarray(a.reshape(n_blk, 128, f).transpose(1, 0, 2))


def _build_nc(debug=False):
    import concourse.bacc as bacc
    import concourse.bass_isa as bass_isa
    import concourse.mybir as mybir
    import concourse.tile as tile

    F32 = mybir.dt.float32
    BF16 = mybir.dt.bfloat16
    AF = mybir.ActivationFunctionType
    SHUF_MASK = list(range(16, 32)) + list(range(0, 16))
    FORD = (1, 3, 0, 2)   # k tensors first

    # Restrict ACT table-set choice to natural_log_exp_and_others (covers
    # Identity/Copy/Ln/Exp) so the whole kernel needs ONE table load instead
    # of alternating set loads (~1.3us each). Names/positions preserved so
    # act_func_set_id indices stay valid.
    _orig_tables = bacc.get_activation_tables

    def _one_set(arch):
        tabs = _orig_tables(arch)
        return {nm: (s if nm == "natural_log_exp_and_others" else set())
                for nm, s in tabs.items()}

    bacc.get_activation_tables = _one_set

    nc = bacc.Bacc("TRN2", target_bir_lowering=False, debug=False,
                   num_devices=N_CORES)

    # ---- DRAM I/O (pre-blocked to SBUF layout host-side) ----
    xT_d = nc.dram_tensor("xTb", [128, 16, 4, 512], BF16, kind="ExternalInput")
    wqk_d = nc.dram_tensor("wqkb", [128, 16, 512], BF16, kind="ExternalInput")
    wv_d = nc.dram_tensor("wvb", [128, 16, 256], BF16, kind="ExternalInput")
    pw_d = nc.dram_tensor("pwb", [128, 2, C], BF16, kind="ExternalInput")
    bqk_d = nc.dram_tensor("bias_qk", [128, 4], F32, kind="ExternalInput")
    cq_d = nc.dram_tensor("cosq", [128, N], BF16, kind="ExternalInput")
    sq_d = nc.dram_tensor("sinq", [128, N], BF16, kind="ExternalInput")
    ck_d = nc.dram_tensor("cosk", [128, N], BF16, kind="ExternalInput")
    sk_d = nc.dram_tensor("sink", [128, N], BF16, kind="ExternalInput")
    eps_d = nc.dram_tensor("epsc", [128, 1], F32, kind="ExternalInput")
    # bf16 partials: halves the output DMA (the host accumulates 8 partials,
    # each bf16 rounding adds ~0.2% incoherent error - budget is 2e-2)
    out_d = nc.dram_tensor("partial", [N, C], BF16, kind="ExternalOutput")
    if debug:
        dbg_qk = [nc.dram_tensor(f"dbg_qk{i}", [128, N], F32, kind="ExternalOutput")
                  for i in range(4)]
        dbg_v = nc.dram_tensor("dbg_v", [128, 16, 256], F32, kind="ExternalOutput")
        dbg_ctx = nc.dram_tensor("dbg_ctx", [128, 2, N], F32, kind="ExternalOutput")

    with tile.TileContext(nc) as tc:
        with (
            tc.tile_pool(name="persist", bufs=1) as pp,
            # rope scratch pools stay open into phase 2: the q-tensor rope of
            # chunks 2-3 is deferred past the phase boundary.
            tc.tile_pool(name="qraw", bufs=8) as qrawp,
            tc.tile_pool(name="scr", bufs=4) as scr,
            tc.tile_pool(name="rbc", bufs=6) as rbcp,
            tc.tile_pool(name="sqp", bufs=3) as sqp,
            tc.tile_pool(name="ssqp", bufs=3) as ssqp,
            tc.tile_pool(name="lnp", bufs=3) as lnp,
        ):
            # resident SBUF tensors
            wqk_sb = pp.tile([128, 16, 512], BF16, name="wqk_sb")
            wv_sb = pp.tile([128, 16, 256], BF16, name="wv_sb")
            pw_sb = pp.tile([128, 2, C], BF16, name="pw_sb")
            bqk_sb = pp.tile([128, 4], F32, name="bqk_sb")
            eps_sb = pp.tile([128, 1], F32, name="eps_sb")
            logq_sb = pp.tile([128, 1], F32, name="logq_sb")
            # -0.25*ln(128): the D^-1/2 attention scale split across q and k
            nc.vector.memset(logq_sb[:], float(-0.25 * np.log(128.0)))
            tab_sb = {nm: pp.tile([128, N], BF16, name=f"tab_{nm}")
                      for nm in ("cq", "sq", "ck", "sk")}
            tab_dram = {"cq": cq_d, "sq": sq_d, "ck": ck_d, "sk": sk_d}

            # final q/k (transposed, rope'd, scaled) and v, ctx
            qk_f = [pp.tile([128, N], BF16, name=f"qkf{i}") for i in range(4)]
            v_sb = pp.tile([128, 16, 256], BF16, name="v_sb")
            ctx_sb = pp.tile([128, 2, N], BF16, name="ctx_sb")

            # table per tensor index: 0:q0 1:k0 2:q1 3:k1
            tab_of = [("cq", "sq"), ("ck", "sk"), ("cq", "sq"), ("ck", "sk")]

            qraw_tiles = {}
            rbcs = {}
            xt_tiles = {}

            def chain_A(c4, f):
                # RMS factor r on all partitions: partition all-reduce of
                # sum-sq, then full-tile Ln/Exp (ACT cost is free-size
                # bound - same as a single row).
                qraw = qraw_tiles[(c4, f)]
                sq = sqp.tile([128, 512], F32, tag="sq", name=f"sq{c4}_{f}")
                nc.vector.tensor_mul(sq[:], qraw[:], qraw[:])
                ssq = ssqp.tile([128, 512], F32, tag="ssq", name=f"ssq{c4}_{f}")
                nc.gpsimd.partition_all_reduce(ssq[:], sq[:], 128,
                                               bass_isa.ReduceOp.add)
                lnr = lnp.tile([128, 512], F32, tag="lnr", name=f"lnr{c4}_{f}")
                nc.scalar.activation(lnr[:], ssq[:], AF.Ln,
                                     scale=1.0 / 128.0, bias=eps_sb[:, 0:1])
                rbc = rbcp.tile([128, 512], BF16, tag="rbc", name=f"rbc{c4}_{f}")
                # r = mean_sq^-1/2 * D^-1/4  (D^-1/2 split across q and k)
                nc.scalar.activation(rbc[:], lnr[:], AF.Exp, scale=-0.5,
                                     bias=logq_sb[:, 0:1])
                rbcs[(c4, f)] = rbc

            def rope_B(c4, f):
                # rotation + scaling, all bf16 (DVE 2x mode on the muls)
                tsl = slice(c4 * 512, (c4 + 1) * 512)
                qraw = qraw_tiles.pop((c4, f))
                cosT = tab_sb[tab_of[f][0]]
                sinT = tab_sb[tab_of[f][1]]
                m1 = scr.tile([128, 512], BF16, tag="m1", name=f"m1_{c4}_{f}")
                nc.vector.tensor_mul(m1[:], qraw[:], cosT[:, tsl])
                sh = scr.tile([128, 512], BF16, tag="sh", name=f"sh{c4}_{f}")
                nc.vector.stream_shuffle(sh[:], qraw[:], SHUF_MASK)
                nc.vector.tensor_mul(sh[:], sh[:], sinT[:, tsl])
                nc.vector.tensor_add(m1[:], m1[:], sh[:])
                nc.vector.tensor_mul(qk_f[f][:, tsl], m1[:], rbcs.pop((c4, f))[:])

            # ---------------- phase 1: QKV + RMS + RoPE ----------------
            # pool lifetimes are managed manually: scope A (chunks 0-2 psum)
            # closes so the attention pools can open while the LAST chunk is
            # still computing; the qc0 score/exp prelude runs inside the
            # phase-1 tail where ACT would otherwise idle.
            # xtall lives on the RIGHT side of the SBUF heap: it outlives the
            # chunk 0-2 PSUM pools but dies before the output-staging pool,
            # which the left stack's LIFO discipline can't express
            xtap = tc.alloc_tile_pool(name="xtall", bufs=2, side="right")
            ps_qk = tc.alloc_tile_pool(name="ps_qk", bufs=4, space="PSUM")
            ps_v = tc.alloc_tile_pool(name="ps_v", bufs=2, space="PSUM")
            if True:
                def drain_qk(c4, qk_ps_of, ford):
                    for f in ford:
                        qraw = qrawp.tile([128, 512], BF16, tag="qraw",
                                          name=f"qraw{c4}_{f}")
                        nc.scalar.activation(qraw[:], qk_ps_of[f][:], AF.Identity,
                                             bias=bqk_sb[:, f:f + 1], scale=1.0)
                        qraw_tiles[(c4, f)] = qraw

                # -- c4=0: fine-grained weight/x DMAs (finest first) so the
                # first matmul starts ~3us in; every later chunk is one DMA.
                xt0 = xtap.tile([128, 16, 512], BF16, tag="xta", name="xta0")
                xt_tiles[0] = xt0
                groups = [(0, 1), (1, 2), (2, 4), (4, 6), (6, 9), (9, 12),
                          (12, 16)]
                for g, (a, b) in enumerate(groups):
                    gsl = slice(a, b)
                    nc.sync.dma_start(wqk_sb[:, gsl, :], wqk_d[:, gsl, :])
                    nc.sync.dma_start(xt0[:, gsl, :], xT_d[:, gsl, 0, :])
                    if g == 3:
                        nc.sync.dma_start(wv_sb[:, 0:8, :], wv_d[:, 0:8, :])
                    elif g == 4:
                        nc.sync.dma_start(wv_sb[:, 8:16, :], wv_d[:, 8:16, :])
                    elif g == 5:
                        nc.sync.dma_start(bqk_sb[:], bqk_d[:])
                        nc.sync.dma_start(eps_sb[:], eps_d[:])

                for c4 in range(3):
                    xta = xt_tiles[c4]
                    # prefetch next token chunk in one DMA
                    nxt = xtap.tile([128, 16, 512], BF16, tag="xta",
                                    name=f"xta{c4 + 1}")
                    nc.sync.dma_start(nxt[:], xT_d[:, :, c4 + 1, :])
                    xt_tiles[c4 + 1] = nxt
                    if c4 == 1:
                        # rope tables: first needed by rope_B(0) late in this
                        # chunk; kept off the chunk-0 critical DMA path
                        for nm in ("ck", "sk", "cq", "sq"):
                            nc.sync.dma_start(tab_sb[nm][:], tab_dram[nm][:])

                    qk_ps = [ps_qk.tile([128, 512], F32, tag="qkps",
                                        name=f"qkps{c4}_{_f}") for _f in range(4)]
                    # [128,1024] = 2 banks, two 256-wide v regions per bank.
                    # Only the first region per bank passes start=True (clears
                    # the whole bank); the second region's first matmul relies
                    # on the cleared has_written bits to overwrite, which is
                    # safe because the PE executes matmuls strictly in program
                    # order.
                    v_ps = ps_v.tile([128, 1024], F32, tag="vps", name=f"vps{c4}")
                    def v_mms(i):
                        for j in range(4):
                            nc.tensor.matmul(v_ps[:, j * 256:(j + 1) * 256],
                                             xta[:, i, j * 128:(j + 1) * 128],
                                             wv_sb[:, i, :],
                                             start=(i == 0 and j % 2 == 0),
                                             stop=(i == 15),
                                             skip_group_check=True)

                    # v matmuls lag behind qk: 2 steps so the chunk-0 start
                    # isn't gated on the wv DMA arriving; 4 steps on chunk 2
                    # so the trailing v work covers this chunk's first PSUM
                    # drain before the last-chunk k blocks reuse its bank
                    lag = 4 if c4 == 2 else 2
                    for i in range(16):
                        for f in range(4):
                            nc.tensor.matmul(qk_ps[f][:],
                                             wqk_sb[:, i, f * 128:(f + 1) * 128],
                                             xta[:, i, :],
                                             start=(i == 0), stop=(i == 15))
                        if i >= lag:
                            v_mms(i - lag)
                    for i in range(16 - lag, 16):
                        v_mms(i)
                    for j in range(4):
                        nc.vector.tensor_copy(v_sb[:, c4 * 4 + j, :],
                                              v_ps[:, j * 256:(j + 1) * 256])
                    # drain in ring order (next chunk's allocs reuse f0 first)
                    drain_qk(c4, qk_ps, (0, 1, 2, 3))
                    # rope the PREVIOUS chunk while this chunk's matmuls run
                    if c4 >= 1:
                        for f in FORD:
                            chain_A(c4 - 1, f)
                        for f in FORD:
                            rope_B(c4 - 1, f)

                # ---- last chunk (c4=3) k-tensor blocks still in scope A via
                # the qk ring: their matmuls cover the chunk-2 drains so the
                # pool handoff below has no idle wall.
                nc.sync.dma_start(pw_sb[:], pw_d[:])
                xta3 = xt_tiles[3]
                for f in FORD:
                    chain_A(2, f)
                for f in (1, 3):
                    rope_B(2, f)
                    t = ps_qk.tile([128, 512], F32, tag="qkps",
                                   name=f"qkps3_{f}")
                    for i in range(16):
                        nc.tensor.matmul(t[:],
                                         wqk_sb[:, i, f * 128:(f + 1) * 128],
                                         xta3[:, i, :],
                                         start=(i == 0), stop=(i == 15))
                    qraw = qrawp.tile([128, 512], BF16, tag="qraw",
                                      name=f"qraw3_{f}")
                    nc.scalar.activation(qraw[:], t[:], AF.Identity,
                                         bias=bqk_sb[:, f:f + 1], scale=1.0)
                    qraw_tiles[(3, f)] = qraw
                    chain_A(3, f)
                    rope_B(3, f)

            # chunks 0-2 PSUM released (LIFO); attention pools take the banks
            ps_v.release()
            ps_qk.release()
            ps_st = tc.alloc_tile_pool(name="ps_st", bufs=2, space="PSUM")
            ps_ctx = tc.alloc_tile_pool(name="ps_ctx", bufs=2, space="PSUM")
            ps_qk3 = tc.alloc_tile_pool(name="ps_qk3", bufs=1, space="PSUM")
            ps_v3 = tc.alloc_tile_pool(name="ps_v3", bufs=1, space="PSUM")
            ep = tc.alloc_tile_pool(name="ep", bufs=12)
            treep = tc.alloc_tile_pool(name="treep", bufs=6)
            esump = tc.alloc_tile_pool(name="esump", bufs=2)
            invp = tc.alloc_tile_pool(name="invp", bufs=2)

            # shared attention emitters (used by the phase-1-tail prelude for
            # q-chunk 0 and by the main phase-2 loop)
            e_tiles = {}
            accs = {}
            ctx_tiles = {}

            def att_st(h, qc, wt):
                qT = qk_f[2 * h]
                kT = qk_f[2 * h + 1]
                qsl = slice(qc * 512, (qc + 1) * 512)
                st = ps_st.tile([128, 1024], F32, tag="st",
                                name=f"st{h}_{qc}_{wt}")
                nc.tensor.matmul(st[:, 0:512],
                                 kT[:, (2 * wt) * 128:(2 * wt + 1) * 128],
                                 qT[:, qsl], start=True, stop=True)
                nc.tensor.matmul(st[:, 512:1024],
                                 kT[:, (2 * wt + 1) * 128:(2 * wt + 2) * 128],
                                 qT[:, qsl], start=True, stop=True,
                                 skip_group_check=True)
                e = ep.tile([128, 1024], BF16, tag="e", name=f"e{h}_{qc}_{wt}")
                # no max subtraction needed: scores are ~N(0,1)
                nc.scalar.activation(e[:], st[:], AF.Exp)
                lst = e_tiles.setdefault((h, qc), [])
                # bf16 running sum for the softmax denominators (replaces PE
                # ones-matmuls); sequential adds keep the post-last-exp
                # latency to one add + fold
                if wt >= 1:
                    s = treep.tile([128, 1024], BF16, tag="tr",
                                   name=f"t_{h}_{qc}_{wt}")
                    nc.vector.tensor_add(s[:], (lst[0] if wt == 1
                                                else accs[(h, qc)])[:], e[:])
                    accs[(h, qc)] = s
                lst.append(e)

            def att_pv(h, qc, wt):
                if wt == 0:
                    ctx_tiles[(h, qc)] = ps_ctx.tile([128, 512], F32,
                                                     tag="ctxps",
                                                     name=f"ctxps{h}_{qc}")
                ctx_ps = ctx_tiles[(h, qc)]
                e = e_tiles[(h, qc)][wt]
                nc.tensor.matmul(ctx_ps[:],
                                 v_sb[:, 2 * wt, h * 128:(h + 1) * 128],
                                 e[:, 0:512], start=(wt == 0), stop=False)
                nc.tensor.matmul(ctx_ps[:],
                                 v_sb[:, 2 * wt + 1, h * 128:(h + 1) * 128],
                                 e[:, 512:1024], start=False, stop=(wt == 7))

            def att_chain(h, qc):
                qsl = slice(qc * 512, (qc + 1) * 512)
                acc = accs.pop((h, qc))
                e_tiles.pop((h, qc))
                esum = esump.tile([128, 512], F32, tag="esum",
                                  name=f"esum{h}_{qc}")
                nc.vector.tensor_add(esum[:], acc[:, 0:512], acc[:, 512:1024])
                sumb = esump.tile([128, 512], F32, tag="sumb",
                                  name=f"sumb{h}_{qc}")
                nc.gpsimd.partition_all_reduce(sumb[:], esum[:], 128,
                                               bass_isa.ReduceOp.add)
                inv = invp.tile([128, 512], F32, tag="inv", name=f"inv{h}_{qc}")
                nc.vector.reciprocal_approx_fast(inv[:], sumb[:])
                nc.vector.tensor_mul(ctx_sb[:, h, qsl],
                                     ctx_tiles.pop((h, qc))[:], inv[:])

            # ---- last chunk (c4=3) q-tensor blocks + v, with the (h0/h1,
            # qc0) score+exp prelude woven in so ACT streams exps while PE
            # finishes the projections. q-tensor RMS/rope of chunks 2-3 are
            # deferred into phase 2 (first read at attention q-chunks 2-3).
            if True:
                def qk3_block(f):
                    t = ps_qk3.tile([128, 512], F32, tag="qk3",
                                    name=f"qkps3_{f}")
                    for i in range(16):
                        nc.tensor.matmul(t[:],
                                         wqk_sb[:, i, f * 128:(f + 1) * 128],
                                         xta3[:, i, :],
                                         start=(i == 0), stop=(i == 15))
                    qraw = qrawp.tile([128, 512], BF16, tag="qraw",
                                      name=f"qraw3_{f}")
                    # q-tensor drains on DVE (GPSIMD can't touch PSUM):
                    # keeps the ACT queue free for the prelude exps
                    nc.vector.tensor_scalar_add(qraw[:], t[:],
                                                bqk_sb[:, f:f + 1])
                    qraw_tiles[(3, f)] = qraw

                def v3_mms(tile, i, jbase):
                    for j in range(2):
                        nc.tensor.matmul(tile[:, j * 256:(j + 1) * 256],
                                         xta3[:, i, (jbase + j) * 128:
                                              (jbase + j + 1) * 128],
                                         wv_sb[:, i, :],
                                         start=(i == 0 and j == 0),
                                         stop=(i == 15),
                                         skip_group_check=True)

                vj01 = ps_v3.tile([128, 512], F32, tag="v3", name="vj01")
                att_st(0, 0, 0)
                att_st(0, 0, 1)
                qk3_block(0)
                att_st(0, 0, 2)
                att_st(0, 0, 3)
                for i in range(8):
                    v3_mms(vj01, i, 0)
                qk3_block(2)
                att_st(0, 0, 4)
                att_st(0, 0, 5)
                for i in range(8, 16):
                    v3_mms(vj01, i, 0)
                att_st(0, 0, 6)
                att_st(0, 0, 7)
                nc.vector.tensor_copy(v_sb[:, 12, :], vj01[:, 0:256])
                nc.scalar.copy(v_sb[:, 13, :], vj01[:, 256:512])
                vj23 = ps_v3.tile([128, 512], F32, tag="v3", name="vj23")
                for wt in range(8):
                    att_st(1, 0, wt)
                    v3_mms(vj23, 2 * wt, 2)
                    v3_mms(vj23, 2 * wt + 1, 2)
                    if wt >= 1:
                        att_pv(0, 0, wt - 1)
                nc.vector.tensor_copy(v_sb[:, 14, :], vj23[:, 0:256])
                nc.scalar.copy(v_sb[:, 15, :], vj23[:, 256:512])

            ps_v3.release()
            ps_qk3.release()
            xtap.release()
            ps_o = tc.alloc_tile_pool(name="ps_o", bufs=2, space="PSUM")
            outp = tc.alloc_tile_pool(name="outp", bufs=3)

            # ------------- phase 2+3: attention + fused projection -------------
            if True:
                # proj work for q-chunk qc as 16 (mt, oc) units, emitted one
                # per attention wide-tile of the NEXT q-chunk so PE/DVE/ACT
                # demand is spread evenly.
                proj_state = {}

                def proj_unit(qc, u, last=False, act_ok=True):
                    mt = 4 * qc + u // 4
                    oc = u % 4
                    msl = slice(mt * 128, (mt + 1) * 128)
                    osl = slice(oc * 512, (oc + 1) * 512)
                    if oc == 0:
                        proj_state[mt] = outp.tile([128, 2048], BF16, tag="ot",
                                                   name=f"ot{mt}")
                    ot = proj_state[mt]
                    po = ps_o.tile([128, 512], F32, tag="po", name=f"po{mt}_{oc}")
                    nc.tensor.matmul(po[:], ctx_sb[:, 0, msl], pw_sb[:, 0, osl],
                                     start=True, stop=False)
                    nc.tensor.matmul(po[:], ctx_sb[:, 1, msl], pw_sb[:, 1, osl],
                                     start=False, stop=True)
                    if last:
                        # final units: alternate drains across the idle
                        # engines; half-row DMAs behind each drained pair
                        if u % 2 == 0:
                            nc.vector.tensor_copy(ot[:, osl], po[:])
                        else:
                            nc.scalar.copy(ot[:, osl], po[:])
                        if oc % 2 == 1:
                            hsl = slice((oc - 1) * 512, (oc + 1) * 512)
                            nc.sync.dma_start(out_d[msl, hsl], ot[:, hsl])
                    else:
                        # steady state: oc 3 on ACT (unless ACT is saturated
                        # by the caller's exps), rest DVE (GPSIMD cannot
                        # read PSUM)
                        if oc == 3 and act_ok:
                            nc.scalar.copy(ot[:, osl], po[:])
                        else:
                            nc.vector.tensor_copy(ot[:, osl], po[:])
                        if oc == 3:
                            nc.sync.dma_start(out_d[msl, :], ot[:])

                def attention(h, qc, slot_cb):
                    # PV lags two wide tiles behind ST so the in-order PE
                    # queue never parks on an exp that hasn't finished; the
                    # first proj slot is held back to the group tail, where
                    # the PE otherwise outruns the last exp of the group
                    for wt in range(8):
                        att_st(h, qc, wt)
                        if wt >= 1:
                            slot_cb()
                        if wt >= 2:
                            att_pv(h, qc, wt - 2)
                    att_pv(h, qc, 6)
                    slot_cb()
                    att_pv(h, qc, 7)
                    att_chain(h, qc)

                # finish q-chunk 0 (scores/exps were emitted in the phase-1
                # tail; only h0's last PV, h1's PVs and the chains remain)
                att_pv(0, 0, 7)
                att_chain(0, 0)
                for wt in range(8):
                    att_pv(1, 0, wt)
                att_chain(1, 0)
                # deferred q-tensor rope of chunks 2-3 (engines are free
                # here; first read at attention q-chunks 2-3)
                rope_B(2, 0)
                chain_A(3, 0)
                rope_B(3, 0)
                rope_B(2, 2)
                chain_A(3, 2)
                rope_B(3, 2)

                def attention_half(h, qc, half, slot_cb):
                    # final-chunk variant: 256-token q half so the last
                    # projection can start after the first half instead of
                    # after the whole chunk. 4 st tiles of [128, 4kt x 256q];
                    # ctx accumulates into a per-head [128,512] bank, one
                    # 256-half per call (half 1 relies on the has_written
                    # clear from half 0's start, like the v_ps trick).
                    qT = qk_f[2 * h]
                    kT = qk_f[2 * h + 1]
                    q0 = qc * 512 + half * 256
                    qsl = slice(q0, q0 + 256)
                    if half == 0:
                        ctx3_ps[h] = ps_ctx.tile([128, 512], F32, tag="ctxps",
                                                 name=f"ctxps3_{h}")
                    ctx_ps = ctx3_ps[h]
                    csl = slice(half * 256, (half + 1) * 256)
                    acc = None
                    eloc = []

                    def pv3(qt):
                        for b in range(4):
                            kt = 4 * qt + b
                            nc.tensor.matmul(ctx_ps[:, csl],
                                             v_sb[:, kt, h * 128:(h + 1) * 128],
                                             eloc[qt][:, b * 256:(b + 1) * 256],
                                             start=(half == 0 and qt == 0 and b == 0),
                                             stop=(qt == 3 and b == 3),
                                             skip_group_check=True)

                    for qt in range(4):
                        st = ps_st.tile([128, 1024], F32, tag="st",
                                        name=f"st3_{h}_{half}_{qt}")
                        for b in range(4):
                            kt = 4 * qt + b
                            nc.tensor.matmul(st[:, b * 256:(b + 1) * 256],
                                             kT[:, kt * 128:(kt + 1) * 128],
                                             qT[:, qsl],
                                             start=(b % 2 == 0), stop=True,
                                             skip_group_check=True)
                        e = ep.tile([128, 1024], BF16, tag="e",
                                    name=f"e3_{h}_{half}_{qt}")
                        nc.scalar.activation(e[:], st[:], AF.Exp)
                        eloc.append(e)
                        if qt >= 1:
                            nacc = treep.tile([128, 1024], BF16, tag="tr",
                                              name=f"t3h_{h}_{half}_{qt}")
                            nc.vector.tensor_add(nacc[:], (eloc[0] if qt == 1
                                                           else acc)[:], e[:])
                            acc = nacc
                        if qt >= 1:
                            slot_cb()
                        if qt >= 1:
                            pv3(qt - 1)
                    slot_cb()
                    pv3(3)
                    f1 = esump.tile([128, 512], F32, tag="esum",
                                    name=f"f1_{h}_{half}")
                    nc.vector.tensor_add(f1[:], acc[:, 0:512], acc[:, 512:1024])
                    f2 = esump.tile([128, 512], F32, tag="sumb",
                                    name=f"f2_{h}_{half}")
                    nc.vector.tensor_add(f2[:, 0:256], f1[:, 0:256],
                                         f1[:, 256:512])
                    nc.gpsimd.partition_all_reduce(f2[:, 256:512],
                                                   f2[:, 0:256], 128,
                                                   bass_isa.ReduceOp.add)
                    inv = invp.tile([128, 512], F32, tag="inv",
                                    name=f"inv3_{h}_{half}")
                    nc.vector.reciprocal_approx_fast(inv[:, 0:256],
                                                     f2[:, 256:512])
                    nc.vector.tensor_mul(ctx_sb[:, h, qsl], ctx_ps[:, csl],
                                         inv[:, 0:256])

                ctx3_ps = {}
                for qc in range(1, 4):
                    slots = [(lambda _qc=qc - 1, _u=u: proj_unit(_qc, _u))
                             for u in range(16)]
                    it = iter(slots)
                    if qc < 3:
                        attention(0, qc, lambda: next(it)())
                        attention(1, qc, lambda: next(it)())
                    else:
                        attention_half(0, 3, 0, lambda: next(it)())
                        attention_half(1, 3, 0, lambda: next(it)())
                        for u in range(0, 8):
                            proj_unit(3, u)
                        attention_half(0, 3, 1, lambda: next(it)())
                        attention_half(1, 3, 1, lambda: next(it)())
                # final 8 units (mt 14-15): borrow the now-idle ps_st banks
                # for 4 extra po slots so six h0-side contractions run during
                # the h1 half's denominator chain; h1-side contractions,
                # drains (alternating engines) and half-row DMAs follow.
                fin_ot = {mt: outp.tile([128, 2048], BF16, tag="ot",
                                        name=f"ot{mt}") for mt in (14, 15)}
                fin_po = []
                for k in range(2):
                    t = ps_st.tile([128, 1024], F32, tag="st", name=f"finpo{k}")
                    fin_po += [t[:, 0:512], t[:, 512:1024]]
                for u in range(12, 14):
                    fin_po.append(ps_o.tile([128, 512], F32, tag="po",
                                            name=f"po_fin{u}")[:])
                for idx, u in enumerate(range(8, 14)):
                    mt, oc = 12 + u // 4, u % 4
                    msl = slice(mt * 128, (mt + 1) * 128)
                    osl = slice(oc * 512, (oc + 1) * 512)
                    nc.tensor.matmul(fin_po[idx], ctx_sb[:, 0, msl],
                                     pw_sb[:, 0, osl], start=True, stop=False,
                                     skip_group_check=True)
                for idx, u in enumerate(range(8, 14)):
                    mt, oc = 12 + u // 4, u % 4
                    msl = slice(mt * 128, (mt + 1) * 128)
                    osl = slice(oc * 512, (oc + 1) * 512)
                    nc.tensor.matmul(fin_po[idx], ctx_sb[:, 1, msl],
                                     pw_sb[:, 1, osl], start=False, stop=True,
                                     skip_group_check=True)
                    ot = fin_ot[mt]
                    if u % 2 == 0:
                        nc.vector.tensor_copy(ot[:, osl], fin_po[idx])
                    else:
                        nc.scalar.copy(ot[:, osl], fin_po[idx])
                    if oc % 2 == 1:
                        hsl = slice((oc - 1) * 512, (oc + 1) * 512)
                        nc.sync.dma_start(out_d[msl, hsl], ot[:, hsl])
                for u in range(14, 16):
                    mt, oc = 15, u % 4
                    msl = slice(mt * 128, (mt + 1) * 128)
                    osl = slice(oc * 512, (oc + 1) * 512)
                    po = ps_o.tile([128, 512], F32, tag="po", name=f"po_fin{u}")
                    nc.tensor.matmul(po[:], ctx_sb[:, 0, msl], pw_sb[:, 0, osl],
                                     start=True, stop=False)
                    nc.tensor.matmul(po[:], ctx_sb[:, 1, msl], pw_sb[:, 1, osl],
                                     start=False, stop=True)
                    if u % 2 == 0:
                        nc.vector.tensor_copy(fin_ot[15][:, osl], po[:])
                    else:
                        nc.scalar.copy(fin_ot[15][:, osl], po[:])
                    if oc % 2 == 1:
                        hsl = slice((oc - 1) * 512, (oc + 1) * 512)
                        nc.sync.dma_start(out_d[msl, hsl], fin_ot[15][:, hsl])

            # LIFO within each (space, side) stack
            ps_o.release()
            ps_ctx.release()
            ps_st.release()
            outp.release()
            invp.release()
            esump.release()
            treep.release()
            ep.release()

            if debug:
                for i in range(4):
                    nc.sync.dma_start(dbg_qk[i][:], qk_f[i][:].bitcast(F32))
                nc.sync.dma_start(dbg_v[:], v_sb[:].bitcast(F32))
                nc.sync.dma_start(dbg_ctx[:], ctx_sb[:].bitcast(F32))

    try:
        nc.compile()
    finally:
        bacc.get_activation_tables = _orig_tables
    return nc


def _host_prep(x, qkv_w, qkv_b, proj_w, proj_b, q_norm_w, k_norm_w, T, H, W):
    perm = _perm_quadrant()
    cos, sin = _rope_tables(T, H, W, D)
    cosq, sinq = _folded_tables(cos, sin, np.asarray(q_norm_w, np.float32), perm)
    cosk, sink = _folded_tables(cos, sin, np.asarray(k_norm_w, np.float32), perm)

    xT = np.asarray(x, np.float32)[0].T          # [C, N]
    qkv_w = np.asarray(qkv_w, np.float32)
    qkv_b = np.asarray(qkv_b, np.float32)
    proj_w = np.asarray(proj_w, np.float32)

    # [C, N] -> [128, 16, 4, 512]: xTb[p, i, c4, q] = xT[i*128+p, c4*512+q]
    xTb = np.ascontiguousarray(
        xT.reshape(16, 128, 4, 512).transpose(1, 0, 2, 3)).astype(NPBF16)

    shared = dict(xTb=xTb,
                  cosq=cosq.astype(NPBF16), sinq=sinq.astype(NPBF16),
                  cosk=cosk.astype(NPBF16), sink=sink.astype(NPBF16),
                  epsc=np.full((128, 1), EPS, np.float32))
    in_maps = []
    for c in range(N_CORES):
        h0 = 2 * c
        wq = [qkv_w[(h0 + j) * D:(h0 + j + 1) * D][perm] for j in range(2)]
        wk = [qkv_w[C + (h0 + j) * D:C + (h0 + j + 1) * D][perm] for j in range(2)]
        bq = [qkv_b[(h0 + j) * D:(h0 + j + 1) * D][perm] for j in range(2)]
        bk = [qkv_b[C + (h0 + j) * D:C + (h0 + j + 1) * D][perm] for j in range(2)]
        wqkT = np.concatenate([wq[0], wk[0], wq[1], wk[1]], axis=0).T  # [C, 512]
        bias_qk = np.stack([bq[0], bk[0], bq[1], bk[1]], axis=1)
        wvT = qkv_w[2 * C + h0 * D:2 * C + (h0 + 2) * D].T             # [C, 256]
        projwT = proj_w[:, h0 * D:(h0 + 2) * D].T                      # [256, C]
        in_maps.append(dict(shared,
                            wqkb=_block_rows(wqkT, 16).astype(NPBF16),
                            wvb=_block_rows(wvT, 16).astype(NPBF16),
                            pwb=_block_rows(projwT, 2).astype(NPBF16),
                            bias_qk=np.ascontiguousarray(bias_qk)))
    v_bias = qkv_b[2 * C:]
    bias_row = (np.asarray(proj_b, np.float32).astype(np.float64)
                + v_bias.astype(np.float64) @ proj_w.astype(np.float64).T)
    return in_maps, bias_row


def kernel(x, qkv_w, qkv_b, proj_w, proj_b, q_norm_w, k_norm_w,
           t_dim, h_dim, w_dim):
    from concourse import bass_utils

    T, H, W = int(t_dim), int(h_dim), int(w_dim)
    if "nc" not in _CACHE:
        _CACHE["nc"] = _build_nc()
    nc = _CACHE["nc"]

    in_maps, bias_row = _host_prep(x, qkv_w, qkv_b, proj_w, proj_b,
                                   q_norm_w, k_norm_w, T, H, W)
    res = bass_utils.run_bass_kernel_spmd(nc, in_maps,
                                          core_ids=list(range(N_CORES)))
    total = np.zeros((N, C), np.float64)
    for r in res.results:
        total += r["partial"]
    out = (total + bias_row[None, :]).astype(np.float32)[None]
    return out
